# revision 1
# baseline (speedup 1.0000x reference)
"""GQA attention (RoPE, full softmax) on 8 TRN2 NeuronCores — v3.

Tensor-parallel over heads: core g owns KV head g and Q heads 4g..4g+3.
Each core computes y_g = concat_h(softmax(Q_h K^T) V_h) @ wo_h^T; the host
sums the 8 bf16 partials in fp32.

Design facts measured on this hardware:
  - A matmul whose stationary operand differs from the previous one pays a
    large weight-reload penalty (~255ns vs ~156ns/FD=512 with reuse), so
    every matmul group keeps the stationary operand for >=2 consecutive
    matmuls under the 8-PSUM-bank budget.
  - ACT activation ops carry ~490ns fixed overhead, so exp() runs on fused
    [128,1024] tiles (a 2-bank PSUM tile written by two FD=512 matmuls).
  - DVE 2-input tensor ops cost ~700ns/[128,512]; softmax row sums
    therefore run on the PE (ones-vector matmuls accumulated in PSUM),
    not as DVE accumulation chains.

Phases:
  A (QKV proj): 3 passes x {2 out-tiles x 4 q-chunks} = 8 banks, c-inner;
    each weight tile feeds 4 consecutive FD=512 matmuls.  x streams bf16;
    the first RES c-tiles stay SBUF-resident so passes 2-3 re-read only
    the tail.  wqkv is preloaded once.  RoPE works on halves (the host
    pre-permutes wq/wk rows); the halves swap is a SBUF->SBUF DMA.
  B (attention): per (chunk-pair, head): S(kt) for both chunks into one
    2-bank PSUM tile, ONE exp -> bf16 P [128,1024], PV(kt-1) with 2x V
    reuse, ones-matmuls accumulate row sums in 2 dedicated PSUM banks.
    PSUM: 4 (S, double-buffered) + 2 (PV acc) + 2 (sums) = 8.
  C (wo): qs-outer / h-mid / e-inner; OT q-block stationary reused 8x,
    8 banks accumulate over h; PSUM->SBUF copies split DVE/ACT; y is bf16
    (host upcasts), halving output DMA.

fp32 PSUM accumulation everywhere; bf16 inputs/activations keep rel err
~5e-3, well under the 2e-2 gate.  exp() without max-subtraction is safe:
scores ~ N(0,1), |s|max ~ 8.
"""

import numpy as np

import concourse.bass as bass
import concourse.mybir as mybir
import concourse.tile as tile
from concourse import bacc
from concourse.bass_utils import run_bass_kernel_spmd
from concourse.masks import make_identity

F32 = mybir.dt.float32
F32R = mybir.dt.float32r
BF16 = mybir.dt.bfloat16
EXP = mybir.ActivationFunctionType.Exp
COPY = mybir.ActivationFunctionType.Copy

DIM, N_HEADS, N_KV_HEADS, HEAD_DIM, SEQ = 4096, 32, 8, 128, 2048
CORES = 8
QH = N_HEADS // CORES  # q heads per core
CHS = 512              # q-chunk size (1 PSUM bank of fp32)
RES = 17               # x c-tiles kept SBUF-resident across phase-A passes


def _body(tc, xT, wqkv, woT, csn, y, dim, seq, qh, phases=3):
    nc = tc.nc
    CT = dim // 128   # contraction tiles over model dim
    KT = seq // 128   # key tiles
    CH = seq // CHS   # q chunks
    HD = HEAD_DIM
    scale = HD ** -0.5

    with tc.tile_pool(name="persist", bufs=1) as persist:
        QT = [persist.tile([128, seq], BF16, name=f"qt{h}", tag=f"qt{h}") for h in range(qh)]
        KTs = persist.tile([128, seq], BF16, tag="kts")
        Vs = persist.tile([128, KT, HD], BF16, tag="vs")
        ones_f = persist.tile([128, 1], F32, tag="ones_f")
        nc.sync.dma_start(out=ones_f, in_=csn[2, :, 0:1])
        onesb = persist.tile([128, 1], BF16, tag="onesb")
        nc.vector.tensor_copy(out=onesb, in_=ones_f)
        ident = persist.tile([128, 128], F32, tag="ident")
        make_identity(nc, ident)
        cs_t = persist.tile([128, seq], F32, tag="cs")
        sn_t = persist.tile([128, seq], F32, tag="sn")

        # ------------- Phase A: QKV projections + RoPE + V transpose -------------
        # passes: 0 -> {q0,q1}, 1 -> {q2,q3}, 2 -> {k,v}
        with (
            tc.tile_pool(name="xres", bufs=1) as xres,
            tc.tile_pool(name="wqa", bufs=1) as wqa,
            tc.tile_pool(name="xs", bufs=5) as xs,
            tc.tile_pool(name="rope", bufs=2) as rp,
            tc.tile_pool(name="pps", bufs=8, space="PSUM") as pps,
        ):
            xresid = xres.tile([128, RES, seq], BF16, tag="xres")
            wqall = wqa.tile([128, CT, (qh + 2) * HD], BF16, tag="wqall")
            WCH = CT // 4
            nc.sync.dma_start(
                out=wqall[:, 0:WCH, :],
                in_=wqkv[0:WCH].rearrange("c p n -> p c n"),
            )

            def rope_batch(ps_list, outs, jsl):
                """RoPE [128,CHS] psum tiles -> bf16 outputs (out = t*cs + swap(t)*sn)."""
                m = len(ps_list)
                tsb = rp.tile([128, 2, CHS], F32, tag="ropesb")
                sw = rp.tile([128, 2, CHS], F32, tag="ropesw")
                for i, t in enumerate(ps_list):
                    if i % 2 == 0:
                        nc.vector.tensor_copy(out=tsb[:, i, :], in_=t)
                    else:
                        nc.scalar.activation(out=tsb[:, i, :], in_=t, func=COPY)
                nc.sync.dma_start(out=sw[0:64, 0:m], in_=tsb[64:128, 0:m])
                nc.sync.dma_start(out=sw[64:128, 0:m], in_=tsb[0:64, 0:m])
                for i, out in enumerate(outs):
                    t1 = rp.tile([128, CHS], F32, tag="rope1")
                    t2 = rp.tile([128, CHS], F32, tag="rope2")
                    nc.vector.tensor_mul(t1, tsb[:, i, :], cs_t[:, jsl])
                    nc.vector.tensor_mul(t2, sw[:, i, :], sn_t[:, jsl])
                    nc.vector.tensor_add(out, t1, t2)

            for p in range(3):
                ps = [
                    [
                        pps.tile([128, CHS], F32, name=f"mm{t}{j}", tag="mm")
                        for j in range(CH)
                    ]
                    for t in range(2)
                ]
                for c in range(CT):
                    if p == 0 and c in (1, 3, 5):
                        i = (c + 1) // 2
                        nc.sync.dma_start(
                            out=wqall[:, i * WCH:(i + 1) * WCH, :],
                            in_=wqkv[i * WCH:(i + 1) * WCH].rearrange("c p n -> p c n"),
                        )
                    if p == 0 and c == 7:
                        nc.sync.dma_start(out=cs_t, in_=csn[0])
                        nc.sync.dma_start(out=sn_t, in_=csn[1])
                    if c < RES:
                        xt = xresid[:, c, :]
                        if p == 0:
                            nc.sync.dma_start(out=xt, in_=xT[c * 128:(c + 1) * 128, :])
                    else:
                        xt = xs.tile([128, seq], BF16, name="xt", tag="xs")
                        nc.sync.dma_start(out=xt, in_=xT[c * 128:(c + 1) * 128, :])
                    for t in range(2):
                        tile_idx = (4 + t) if p == 0 else (2 * (p - 1) + t)
                        w_sl = slice(tile_idx * HD, (tile_idx + 1) * HD)
                        for j in range(CH):
                            nc.tensor.matmul(
                                ps[t][j],
                                lhsT=wqall[:, c, w_sl],
                                rhs=xt[:, j * CHS:(j + 1) * CHS],
                                start=(c == 0),
                                stop=(c == CT - 1),
                            )
                # post-pass: rope / V transpose
                for j in range(CH):
                    jsl = slice(j * CHS, (j + 1) * CHS)
                    if p > 0:
                        rope_batch(
                            [ps[0][j], ps[1][j]],
                            [QT[2 * (p - 1)][:, jsl], QT[2 * p - 1][:, jsl]], jsl,
                        )
                    else:
                        rope_batch([ps[0][j]], [KTs[:, jsl]], jsl)
                        vt_sb = rp.tile([128, CHS], F32, tag="vtsb")
                        nc.scalar.activation(out=vt_sb, in_=ps[1][j], func=COPY)
                        for b in range(CHS // HD):
                            trp = pps.tile([128, HD], F32, name="trp", tag="mm")
                            nc.tensor.transpose(trp, vt_sb[:, b * HD:(b + 1) * HD], ident)
                            nc.vector.tensor_copy(
                                out=Vs[:, j * (CHS // HD) + b, :], in_=trp
                            )

        if phases == 1:
            nc.sync.dma_start(out=y[0:128, 0:seq], in_=KTs)
            return

        # ---------------- Phase B: attention per (chunk-pair, head) ----------------
        with (
            tc.tile_pool(name="wo", bufs=1) as wop,
            tc.tile_pool(name="otp", bufs=1) as otp,
        ):
            # preload wo during phase B (used in phase C)
            wo_r = wop.tile([128, qh, dim], BF16)
            for h in range(qh):
                nc.sync.dma_start(out=wo_r[:, h, :], in_=woT[h])
            OT = [otp.tile([128, seq], BF16, name=f"ot{h}", tag=f"ot{h}") for h in range(qh)]

            with (
                tc.tile_pool(name="pp", bufs=6) as pp,
                tc.tile_pool(name="sm", bufs=3) as sm,
                tc.tile_pool(name="sps", bufs=2, space="PSUM") as sps,
                tc.tile_pool(name="aps", bufs=1, space="PSUM") as aps,
            ):
                for jp in range(CH // 2):
                    for h in range(qh):
                        opst = [
                            aps.tile([128, CHS], F32, name=f"ops{i}", tag=f"ops{i}")
                            for i in range(2)
                        ]
                        sums = [
                            aps.tile([1, CHS], F32, name=f"sums{i}", tag=f"sums{i}")
                            for i in range(2)
                        ]
                        pts = [None] * KT
                        for kt in range(KT):
                            st = sps.tile([128, 2, CHS], F32, name="st", tag="st")
                            for i in range(2):
                                j = 2 * jp + i
                                nc.tensor.matmul(
                                    st[:, i, :],
                                    lhsT=KTs[:, kt * 128:(kt + 1) * 128],
                                    rhs=QT[h][:, j * CHS:(j + 1) * CHS],
                                    start=True,
                                    stop=True,
                                )
                            pt = pp.tile([128, 2, CHS], BF16, name="pt", tag="pt")
                            nc.scalar.activation(out=pt, in_=st, func=EXP, scale=scale)
                            pts[kt] = pt
                            # PV and the row-sum ones-matmuls both lag one kt
                            # so the in-order PE queue never waits on exp(kt)
                            if kt > 0:
                                for i in range(2):
                                    nc.tensor.matmul(
                                        opst[i],
                                        lhsT=Vs[:, kt - 1, :],
                                        rhs=pts[kt - 1][:, i, :],
                                        start=(kt - 1 == 0),
                                        stop=False,
                                    )
                            if kt > 1 and kt % 2 == 0:
                                # batched 2-kt-behind: one ones load per 4 mms
                                for dk in (2, 1):
                                    for i in range(2):
                                        nc.tensor.matmul(
                                            sums[i],
                                            lhsT=onesb,
                                            rhs=pts[kt - dk][:, i, :],
                                            start=(kt - dk == 0),
                                            stop=False,
                                        )
                        for i in range(2):
                            nc.tensor.matmul(
                                opst[i],
                                lhsT=Vs[:, KT - 1, :],
                                rhs=pts[KT - 1][:, i, :],
                                start=False,
                                stop=True,
                            )
                        for dk in (2, 1):
                            for i in range(2):
                                nc.tensor.matmul(
                                    sums[i],
                                    lhsT=onesb,
                                    rhs=pts[KT - dk][:, i, :],
                                    start=False,
                                    stop=(dk == 1),
                                )
                        # fused double-width norm tail: one recip + one
                        # broadcast for both chunks (sums stay in 2 banks)
                        ssb = sm.tile([1, 2, CHS], F32, tag="ssb")
                        nc.scalar.activation(out=ssb[:, 0, :], in_=sums[0], func=COPY)
                        nc.vector.tensor_copy(out=ssb[:, 1, :], in_=sums[1])
                        rec = sm.tile([1, 2, CHS], F32, tag="rec")
                        nc.vector.reciprocal(rec, ssb)
                        rb = sm.tile([128, 2, CHS], F32, tag="rb")
                        nc.gpsimd.partition_broadcast(rb, rec)
                        for i in range(2):
                            j = 2 * jp + i
                            jsl = slice(j * CHS, (j + 1) * CHS)
                            nc.vector.tensor_mul(OT[h][:, jsl], opst[i], rb[:, i, :])

            if phases == 2:
                for h in range(qh):
                    nc.sync.dma_start(
                        out=y[h * 128:(h + 1) * 128, 0:seq], in_=OT[h]
                    )
                return

            # ---------------- Phase C: output projection ----------------
            with (
                tc.tile_pool(name="ysb", bufs=2) as ysb,
                tc.tile_pool(name="yps", bufs=1, space="PSUM") as yps,
            ):
                QS = seq // 128
                ECH = dim // 512
                for qs in range(QS):
                    ybs = [
                        yps.tile([128, 512], F32, name=f"yb{e}", tag=f"yb{e}")
                        for e in range(ECH)
                    ]
                    ystage = ysb.tile([128, dim], BF16, tag="yt")
                    for h in range(qh):
                        for e in range(ECH):
                            nc.tensor.matmul(
                                ybs[e],
                                lhsT=OT[h][:, qs * 128:(qs + 1) * 128],
                                rhs=wo_r[:, h, e * 512:(e + 1) * 512],
                                start=(h == 0),
                                stop=(h == qh - 1),
                            )
                            if h == qh - 1:
                                # emit the drain right after each stop so it
                                # overlaps the remaining accumulations
                                esl = slice(e * 512, (e + 1) * 512)
                                if e < 5:
                                    nc.vector.tensor_copy(
                                        out=ystage[:, esl], in_=ybs[e]
                                    )
                                else:
                                    nc.scalar.activation(
                                        out=ystage[:, esl], in_=ybs[e], func=COPY,
                                    )
                    nc.sync.dma_start(out=y[qs * 128:(qs + 1) * 128, :], in_=ystage)


def build_nc(dim=DIM, seq=SEQ, qh=QH, repeat=1, phases=3):
    ct = dim // 128
    nc = bacc.Bacc("TRN2", target_bir_lowering=False, debug=False)
    xT = nc.dram_tensor("xT", [dim, seq], BF16, kind="ExternalInput").ap()
    wqkv = nc.dram_tensor(
        "wqkv", [ct, 128, (qh + 2) * HEAD_DIM], BF16, kind="ExternalInput"
    ).ap()
    woT = nc.dram_tensor("woT", [qh, HEAD_DIM, dim], BF16, kind="ExternalInput").ap()
    csn = nc.dram_tensor("csn", [3, 128, seq], F32, kind="ExternalInput").ap()
    y = nc.dram_tensor("y", [seq, dim], BF16, kind="ExternalOutput").ap()
    with tile.TileContext(nc) as tc:
        for _ in range(repeat):
            _body(tc, xT, wqkv, woT, csn, y, dim, seq, qh, phases=phases)
    nc.compile()
    return nc


def make_in_maps(x, freqs, wq, wk, wv, wo, cores=CORES):
    """Host-side sharding: returns list of per-core input dicts."""
    import ml_dtypes

    bf = ml_dtypes.bfloat16
    dim = x.shape[1]
    seq = x.shape[0]
    hd = HEAD_DIM
    n_heads = wq.shape[0] // hd
    n_kv = wk.shape[0] // hd
    qh = n_heads // cores
    ct = dim // 128

    perm = np.concatenate([np.arange(0, hd, 2), np.arange(1, hd, 2)])
    cos = np.cos(freqs).T.astype(np.float32)  # [64, S]
    sin = np.sin(freqs).T.astype(np.float32)
    csn = np.stack(
        [
            np.concatenate([cos, cos], axis=0),
            np.concatenate([-sin, sin], axis=0),
            np.ones((128, seq), np.float32),
        ]
    ).astype(np.float32)  # [3, 128, S]

    xT = np.ascontiguousarray(x.T.astype(bf))  # [dim, seq] bf16

    wq_r = wq.reshape(n_heads, hd, dim)
    wk_r = wk.reshape(n_kv, hd, dim)
    wv_r = wv.reshape(n_kv, hd, dim)

    in_maps = []
    for g in range(cores):
        wq_g = wq_r[g * qh:(g + 1) * qh][:, perm, :]  # [qh, 128, dim]
        wk_g = wk_r[g][perm, :]                       # [128, dim]
        wv_g = wv_r[g]                                # [128, dim]
        wq_t = (
            wq_g.reshape(qh, hd, ct, 128).transpose(2, 3, 0, 1).reshape(ct, 128, qh * hd)
        )
        wk_t = wk_g.reshape(hd, ct, 128).transpose(1, 2, 0)  # [ct, 128, 128]
        wv_t = wv_g.reshape(hd, ct, 128).transpose(1, 2, 0)
        wqkv_g = np.ascontiguousarray(
            np.concatenate([wq_t, wk_t, wv_t], axis=2), dtype=bf
        )
        wo_g = wo[:, g * qh * hd:(g + 1) * qh * hd]   # [dim, qh*128]
        woT_g = np.ascontiguousarray(wo_g.T.reshape(qh, hd, dim), dtype=bf)
        in_maps.append({"xT": xT, "wqkv": wqkv_g, "woT": woT_g, "csn": csn})
    return in_maps


_NC_CACHE = {}


def kernel(x, freqs, wq, wk, wv, wo):
    x = np.asarray(x, dtype=np.float32)
    freqs = np.asarray(freqs, dtype=np.float32)
    wq = np.asarray(wq, dtype=np.float32)
    wk = np.asarray(wk, dtype=np.float32)
    wv = np.asarray(wv, dtype=np.float32)
    wo = np.asarray(wo, dtype=np.float32)

    key = (DIM, SEQ, QH)
    if key not in _NC_CACHE:
        _NC_CACHE[key] = build_nc(DIM, SEQ, QH)
    nc = _NC_CACHE[key]

    in_maps = make_in_maps(x, freqs, wq, wk, wv, wo, CORES)
    res = run_bass_kernel_spmd(nc, in_maps, list(range(CORES)))
    parts = [np.asarray(res.results[g]["y"], dtype=np.float32) for g in range(CORES)]
    return np.sum(np.stack(parts), axis=0, dtype=np.float32)


if __name__ == "__main__":
    import reference

    inputs = reference.setup_inputs()
    out = kernel(**{k: np.asarray(v) for k, v in inputs.items()})
    print("kernel out", out.shape, out.dtype)

